# revision 5
# baseline (speedup 1.0000x reference)
"""Linformer attention Trainium2 kernel (8-core SPMD, batch x head-group sharded).

Sharding: core c handles batch b = c//2 and heads [8*(c%2), 8*(c%2)+8).
Each core computes a partial output (contribution of its 8 heads to its batch);
the host sums the two partials per batch and adds b_out.

Math per core (b, heads hs..hs+8), exploiting the Linformer low-rank structure:
  xEF  = EF^T @ x_b                  (128 x 1024 fp32 psum; E rows 0:64, F 64:128)
  klrT = Wk-chunks^T @ xET-chunks    ([c, k] layout directly; + rank-1 bias on copy)
  kbd  = pair-stacked block-diag klrT (fp16)        vbd likewise from vlrT
  M    = WqT-tiles^T @ kbd           (m_sb [d, hk], fp16)
  dcorrT = kbd^T @ bq                (per-hk dots bias -> folded into exp bias)
  vw   = vbd^T @ WoB-tiles           (vw_sb [hk, e], fp16)
  per 512-row superblock (transposed pass B; no attn transposes needed):
    dotsT_t = sum_j m_sb[:,j,t]^T @ xT-tiles       ([hk, r] in psum)
    expT_t  = Exp(0.125*dotsT_t + bias_t)          (ACT, per-partition bias, bf16)
    sums_t  = blockones^T @ expT_t                 (per-head partition sums, bcast)
    attnT_t = expT_t / sums_t                      (DVE divide, fp16)
    out_q  += attnT_t-tiles^T @ vw_sb              (fp16 matmul, fp32 accum)
The q/k chain stays fp16-operand (fp32 accumulation) end to end; softmax uses a
constant shift (exp args stay in bf16 range; see baseline analysis).
"""

import sys

import numpy as np

try:
    import concourse.bass as bass  # noqa: F401
except ImportError:
    sys.path.insert(0, "/opt/trn_rl_repo")

from contextlib import ExitStack

import concourse.bass as bass
import concourse.tile as tile
from concourse import bacc, mybir
from concourse.bass_utils import run_bass_kernel_spmd

N, B, DIM, H, K, DH = 4096, 4, 1024, 16, 64, 64
NH = 8           # heads per core
QC = NH * DH     # 512, per-core q/k/v column span
NCORES = 8
NCHUNK = N // 128      # 32 row chunks
NSUPER = 8             # 8 superblocks of 512 rows
FP32 = mybir.dt.float32
FP16 = mybir.dt.float16
BF16 = mybir.dt.bfloat16

_PROG_CACHE = {}


def build_program():
    if "nc" in _PROG_CACHE:
        return _PROG_CACHE["nc"]
    nc = bacc.Bacc("TRN2", target_bir_lowering=False, debug=False)

    ident = nc.dram_tensor("ident", [128, 128], FP16, kind="ExternalInput")
    bones = nc.dram_tensor("bones", [128, 128], BF16, kind="ExternalInput")
    EFp = nc.dram_tensor("EFp", [128, NCHUNK * 2 * K], FP16, kind="ExternalInput")
    x_nat = nc.dram_tensor("x_nat", [N, DIM], FP16, kind="ExternalInput")
    Wk = nc.dram_tensor("Wk", [DIM, QC], FP16, kind="ExternalInput")
    Wv = nc.dram_tensor("Wv", [DIM, QC], FP16, kind="ExternalInput")
    WqT = nc.dram_tensor("WqT", [QC, DIM], FP16, kind="ExternalInput")
    WoB = nc.dram_tensor("WoB", [QC, DIM], FP16, kind="ExternalInput")
    bqp = nc.dram_tensor("bqp", [128, 4], FP16, kind="ExternalInput")
    r1k = nc.dram_tensor("r1k", [128, 4 * K], FP32, kind="ExternalInput")
    r1v = nc.dram_tensor("r1v", [128, 4 * K], FP32, kind="ExternalInput")
    xT = nc.dram_tensor("xT", [DIM, N], FP16, kind="ExternalInput")
    out_p = nc.dram_tensor("out_p", [N, DIM], FP16, kind="ExternalOutput")

    with tile.TileContext(nc) as tc, ExitStack() as ctx:
        singles = ctx.enter_context(tc.tile_pool(name="singles", bufs=1))

        # --- prologue: tiny DMAs first, then PE warmup under the x stream ---
        ident_t = singles.tile([128, 128], FP16)
        nc.sync.dma_start(ident_t[:], ident[:])
        bones_t = singles.tile([128, 128], BF16)
        nc.sync.dma_start(bones_t[:], bones[:])
        ef_t = singles.tile([128, NCHUNK, 2 * K], FP16)
        nc.sync.dma_start(ef_t[:], EFp[:].rearrange("p (i k) -> p i k", i=NCHUNK))

        wm_src = singles.tile([128, 512], FP16)
        nc.vector.memset(wm_src[:], 1.0)
        with tc.tile_pool(name="warm", bufs=1, space="PSUM") as warm_pool:
            wm_ps = warm_pool.tile([128, 512], FP32)
            for _ in range(14):
                nc.tensor.matmul(wm_ps[:], ident_t[:], wm_src[:],
                                 start=True, stop=True)

        # ---------------- Pass A: xEF = EF^T @ x  (fp32 psum) ----------------
        a2sb = ctx.enter_context(tc.tile_pool(name="a2sb", bufs=1))
        xe16 = a2sb.tile([64, DIM], FP16)
        xf16 = a2sb.tile([64, DIM], FP16)
        with tc.tile_pool(name="xef_ps", bufs=1, space="PSUM") as xef_pool:
            xef_ps = xef_pool.tile([128, DIM], FP32)
            with tc.tile_pool(name="xa", bufs=8) as xa_pool:
                for i in range(NCHUNK):
                    x_t = xa_pool.tile([128, DIM], FP16)
                    nc.sync.dma_start(x_t[:], x_nat[i * 128:(i + 1) * 128, :])
                    for f in (0, 512):
                        nc.tensor.matmul(xef_ps[:, f:f + 512], ef_t[:, i, :],
                                         x_t[:, f:f + 512],
                                         start=(i == 0), stop=(i == NCHUNK - 1))
            nc.vector.tensor_copy(xe16[:], xef_ps[0:64, :])
            nc.scalar.copy(xf16[:], xef_ps[64:128, :])

        # weight / constant DMAs (queued behind the x chunks; land during A2)
        wk_t = singles.tile([128, 8, QC], FP16)
        nc.sync.dma_start(wk_t[:], Wk[:].rearrange("(j p) c -> p j c", p=128))
        wv_t = singles.tile([128, 8, QC], FP16)
        nc.sync.dma_start(wv_t[:], Wv[:].rearrange("(j p) c -> p j c", p=128))
        r1k_t = singles.tile([128, 4, K], FP32)
        nc.sync.dma_start(r1k_t[:], r1k[:].rearrange("p (t k) -> p t k", t=4))
        r1v_t = singles.tile([128, 4, K], FP32)
        nc.sync.dma_start(r1v_t[:], r1v[:].rearrange("p (t k) -> p t k", t=4))
        bqp_t = singles.tile([128, 4], FP16)
        nc.sync.dma_start(bqp_t[:], bqp[:])
        wqt_t = singles.tile([128, 4, DIM], FP16)
        nc.sync.dma_start(wqt_t[:], WqT[:].rearrange("(t p) c -> p t c", p=128))
        wob_t = singles.tile([128, 4, DIM], FP16)
        nc.sync.dma_start(wob_t[:], WoB[:].rearrange("(t p) c -> p t c", p=128))

        # ---------------- Pass A2: klrT/vlrT -> kbd/vbd -> M, dcorrT, vw ----
        kbd = a2sb.tile([128, 4, 128], FP16)
        nc.vector.memset(kbd[:], 0.0)
        vbd = a2sb.tile([128, 4, 128], FP16)
        nc.vector.memset(vbd[:], 0.0)
        m_sb = a2sb.tile([128, 8, QC], FP16)
        vw_sb = a2sb.tile([128, 4, DIM], FP16)
        expbias = a2sb.tile([128, 4], FP32)

        with tc.tile_pool(name="a2ps", bufs=2, space="PSUM") as a2ps:
            # transpose xE/xF: [64, 1024] -> 8 chunks of [128, 64] each
            xet_sb = a2sb.tile([128, 8, K], FP16)
            xft_sb = a2sb.tile([128, 8, K], FP16)
            for (src, dst) in ((xe16, xet_sb), (xf16, xft_sb)):
                tp = a2ps.tile([128, 8, K], FP16, tag="xt_ps")
                for j in range(8):
                    nc.tensor.transpose(
                        tp[:, j, :], src[:, j * 128:(j + 1) * 128],
                        ident_t[0:64, 0:64],
                    )
                nc.vector.tensor_copy(dst[:], tp[:])

            # klrT/vlrT tiles [c_pair, k] directly; + rank-1 bias on the copy
            for (w_t, xt, r1, dst) in (
                (wk_t, xet_sb, r1k_t, kbd),
                (wv_t, xft_sb, r1v_t, vbd),
            ):
                for t in range(4):
                    lp = a2ps.tile([128, K], FP32, tag="lr_ps")
                    for j in range(8):
                        nc.tensor.matmul(
                            lp[:], w_t[:, j, t * 128:(t + 1) * 128], xt[:, j, :],
                            start=(j == 0), stop=(j == 7),
                        )
                    nc.vector.tensor_add(out=dst[0:64, t, 0:64],
                                         in0=lp[0:64, :], in1=r1[0:64, t, :])
                    nc.vector.tensor_add(out=dst[64:128, t, 64:128],
                                         in0=lp[64:128, :], in1=r1[64:128, t, :])

            # M tiles: m_sb[p, j, hk] = M[j*128+p, hk] = (Wq klr^T)[d, hk]
            for j in range(8):
                m_ps = a2ps.tile([128, QC], FP32, tag="big")
                for t in range(4):
                    nc.tensor.matmul(
                        m_ps[:, t * 128:(t + 1) * 128],
                        wqt_t[:, t, j * 128:(j + 1) * 128],
                        kbd[:, t, :],
                        start=True, stop=True,
                    )
                if j % 2 == 0:
                    nc.vector.tensor_copy(m_sb[:, j, :], m_ps[:])
                else:
                    nc.scalar.copy(m_sb[:, j, :], m_ps[:])

            # dcorrT[hk] = bq . klr[hk-row]; exp bias = 0.125*dcorrT - 80
            dc_big = a2ps.tile([128, 512], FP32, tag="big")
            dc_ps = dc_big[:, 0:4]
            for t in range(4):
                nc.tensor.matmul(dc_ps[:, t:t + 1], kbd[:, t, :],
                                 bqp_t[:, t:t + 1], start=True, stop=True)
            nc.scalar.activation(out=expbias[:], in_=dc_ps[:],
                                 func=mybir.ActivationFunctionType.Copy,
                                 bias=-80.0, scale=0.125)

            # vw: pair-stacked (vlr_h @ Wout_h) in fp16
            for t in range(4):
                for f in (0, 512):
                    vw_ps = a2ps.tile([128, 512], FP32, tag="big")
                    nc.tensor.matmul(vw_ps[:], vbd[:, t, :],
                                     wob_t[:, t, f:f + 512],
                                     start=True, stop=True)
                    if f == 0:
                        nc.vector.tensor_copy(vw_sb[:, t, f:f + 512], vw_ps[:])
                    else:
                        nc.scalar.copy(vw_sb[:, t, f:f + 512], vw_ps[:])

        # ---------------- Pass B: dotsT -> softmax -> out --------------------
        xt_pool = ctx.enter_context(tc.tile_pool(name="xt", bufs=3))
        exp_pool = ctx.enter_context(tc.tile_pool(name="expp", bufs=5))
        attn_pool = ctx.enter_context(tc.tile_pool(name="attn", bufs=2))
        osb_pool = ctx.enter_context(tc.tile_pool(name="osb", bufs=3))
        dots_pool = ctx.enter_context(tc.tile_pool(name="dots", bufs=2, space="PSUM"))
        sums_pool = ctx.enter_context(tc.tile_pool(name="sums", bufs=2, space="PSUM"))
        out_ps_pool = ctx.enter_context(tc.tile_pool(name="outps", bufs=2, space="PSUM"))

        xts_tiles = [None] * NSUPER
        attn_tiles = [None] * NSUPER

        def front_half(s):
            """dotsT -> exp -> sums -> attnT for superblock s."""
            xts = xt_pool.tile([128, 8, 512], FP16)
            nc.sync.dma_start(
                xts[:],
                xT[:, s * 512:(s + 1) * 512].rearrange("(j p) n -> p j n", p=128),
            )
            xts_tiles[s] = xts
            attnT = attn_pool.tile([128, 4, 512], FP16)
            attn_tiles[s] = attnT
            for t in range(4):
                dots_ps = dots_pool.tile([128, 512], FP32)
                for j in range(8):
                    nc.tensor.matmul(
                        dots_ps[:], m_sb[:, j, t * 128:(t + 1) * 128],
                        xts[:, j, :],
                        start=(j == 0), stop=(j == 7),
                    )
                expT = exp_pool.tile([128, 512], BF16)
                nc.scalar.activation(
                    out=expT[:], in_=dots_ps[:],
                    func=mybir.ActivationFunctionType.Exp,
                    scale=0.125, bias=expbias[:, t:t + 1],
                )
                sums_ps = sums_pool.tile([128, 512], FP32)
                nc.tensor.matmul(sums_ps[:], bones_t[:], expT[:],
                                 start=True, stop=True)
                rec = exp_pool.tile([128, 512], FP32, tag="rec")
                nc.vector.reciprocal(rec[:], sums_ps[:])
                nc.vector.tensor_mul(out=attnT[:, t, :], in0=expT[:], in1=rec[:])

        def back_half(s):
            """attnT @ vw -> out rows for superblock s."""
            attnT = attn_tiles[s]
            for q in range(4):
                out_ps = out_ps_pool.tile([128, DIM], FP32)
                for f in (0, 512):
                    for t in range(4):
                        nc.tensor.matmul(
                            out_ps[:, f:f + 512],
                            attnT[:, t, q * 128:(q + 1) * 128],
                            vw_sb[:, t, f:f + 512],
                            start=(t == 0), stop=(t == 3),
                        )
                out_sb = osb_pool.tile([128, DIM], FP16)
                nc.vector.tensor_copy(out_sb[:, 0:512], out_ps[:, 0:512])
                nc.scalar.copy(out_sb[:, 512:1024], out_ps[:, 512:1024])
                i = s * 4 + q
                nc.gpsimd.dma_start(out_p[i * 128:(i + 1) * 128, :], out_sb[:])

        # software-pipeline by one superblock so the PE never waits on softmax
        front_half(0)
        for s in range(1, NSUPER):
            front_half(s)
            back_half(s - 1)
        back_half(NSUPER - 1)

    nc.finalize()
    _PROG_CACHE["nc"] = nc
    return nc


def shard_inputs(x, E, F, W_qkv, b_qkv, W_out, b_out):
    """Host-side prep: slice / transpose / cast per core."""
    x = np.asarray(x, dtype=np.float32)
    E = np.asarray(E, dtype=np.float32)
    F = np.asarray(F, dtype=np.float32)
    W_qkv = np.asarray(W_qkv, dtype=np.float32)
    b_qkv = np.asarray(b_qkv, dtype=np.float32)
    W_out = np.asarray(W_out, dtype=np.float32)

    sE = E.sum(0).astype(np.float32)   # [K]
    sF = F.sum(0).astype(np.float32)
    # EF pre-chunked: EFp[p, i*2K + k] = EF[i*128 + p, k]
    EF = np.concatenate([E, F], axis=1).astype(np.float16)       # [N, 2K]
    EFp = np.ascontiguousarray(
        EF.reshape(NCHUNK, 128, 2 * K).transpose(1, 0, 2).reshape(128, -1))

    ident = np.eye(128, dtype=np.float16)
    bones = np.zeros((128, 128), np.float32)
    bones[:64, :64] = 1.0
    bones[64:, 64:] = 1.0
    import ml_dtypes
    bones = bones.astype(ml_dtypes.bfloat16)

    in_maps = []
    xb_cache = {}
    for c in range(NCORES):
        b, hg = c // 2, c % 2
        hs = NH * hg
        if b not in xb_cache:
            xb16 = np.ascontiguousarray(x[:, b, :]).astype(np.float16)
            xT16 = np.ascontiguousarray(xb16.T)
            xb_cache[b] = (xb16, xT16)
        xb16, xT16 = xb_cache[b]

        qcols = slice(hs * DH, (hs + NH) * DH)
        kcols = slice(DIM + hs * DH, DIM + (hs + NH) * DH)
        vcols = slice(2 * DIM + hs * DH, 2 * DIM + (hs + NH) * DH)

        bq = b_qkv[qcols]                      # [512]
        bqp16 = np.ascontiguousarray(
            bq.reshape(4, 128).T).astype(np.float16)             # [128, 4]
        bk = b_qkv[kcols]
        bv = b_qkv[vcols]
        # r1kT[p, t*K + k] = bk[t*128 + p] * sE[k]
        r1kT = np.ascontiguousarray(
            (bk.reshape(4, 128)[:, :, None] * sE[None, None, :])
            .transpose(1, 0, 2).reshape(128, 4 * K))
        r1vT = np.ascontiguousarray(
            (bv.reshape(4, 128)[:, :, None] * sF[None, None, :])
            .transpose(1, 0, 2).reshape(128, 4 * K))

        in_maps.append({
            "ident": ident,
            "bones": bones,
            "EFp": EFp,
            "x_nat": xb16,
            "Wk": W_qkv[:, kcols].astype(np.float16),
            "Wv": W_qkv[:, vcols].astype(np.float16),
            "WqT": np.ascontiguousarray(W_qkv[:, qcols].T).astype(np.float16),
            "WoB": W_out[hs * DH:(hs + NH) * DH, :].astype(np.float16),
            "bqp": bqp16,
            "r1k": r1kT.astype(np.float32),
            "r1v": r1vT.astype(np.float32),
            "xT": xT16,
        })
    return in_maps


def kernel_impl(inputs, trace=False, **run_kwargs):
    nc = build_program()
    in_maps = shard_inputs(
        inputs["x"], inputs["E"], inputs["F"], inputs["W_qkv"],
        inputs["b_qkv"], inputs["W_out"], inputs["b_out"],
    )
    res = run_bass_kernel_spmd(nc, in_maps, list(range(NCORES)),
                               trace=trace, **run_kwargs)
    b_out = np.asarray(inputs["b_out"], dtype=np.float32)
    out = np.empty((N, B, DIM), np.float32)
    for b in range(B):
        out[:, b, :] = (res.results[2 * b]["out_p"].astype(np.float32)
                        + res.results[2 * b + 1]["out_p"].astype(np.float32)
                        + b_out)
    return out, res


def kernel(**inputs):
    out, _ = kernel_impl(inputs)
    return out


# revision 6
# speedup vs baseline: 1.4318x; 1.4318x over previous
"""Linformer attention Trainium2 kernel (8-core SPMD, batch x head-group sharded).

Sharding: core c handles batch b = c//2 and heads [8*(c%2), 8*(c%2)+8).
Each core computes a partial output (contribution of its 8 heads to its batch);
the host sums the two partials per batch and adds b_out.

Math per core (b, heads hs..hs+8), exploiting the Linformer low-rank structure:
  xEF  = EF^T @ x_b                  (128 x 1024 fp32 psum; E rows 0:64, F 64:128)
  klrT = Wk-chunks^T @ xET-chunks    ([c, k] layout directly; + rank-1 bias on copy)
  kbd  = pair-stacked block-diag klrT (fp16)        vbd likewise from vlrT
  M    = WqT-tiles^T @ kbd           (m_sb [d, hk], fp16)
  dcorrT = kbd^T @ bq                (per-hk dots bias -> folded into exp bias)
  vw   = vbd^T @ WoB-tiles           (vw_sb [hk, e], fp16)
  per 512-row superblock (transposed pass B; no attn transposes needed):
    dotsT_t = sum_j m_sb[:,j,t]^T @ xT-tiles       ([hk, r] in psum)
    expT_t  = Exp(0.125*dotsT_t + bias_t)          (ACT, per-partition bias, bf16)
    sums_t  = blockones^T @ expT_t                 (per-head partition sums, bcast)
    attnT_t = expT_t / sums_t                      (DVE divide, fp16)
    out_q  += attnT_t-tiles^T @ vw_sb              (fp16 matmul, fp32 accum)
The q/k chain stays fp16-operand (fp32 accumulation) end to end; softmax uses a
constant shift (exp args stay in bf16 range; see baseline analysis).
"""

import sys

import numpy as np

try:
    import concourse.bass as bass  # noqa: F401
except ImportError:
    sys.path.insert(0, "/opt/trn_rl_repo")

from contextlib import ExitStack

import concourse.bass as bass
import concourse.tile as tile
from concourse import bacc, mybir
from concourse.bass_utils import run_bass_kernel_spmd

N, B, DIM, H, K, DH = 4096, 4, 1024, 16, 64, 64
NH = 8           # heads per core
QC = NH * DH     # 512, per-core q/k/v column span
NCORES = 8
NCHUNK = N // 128      # 32 row chunks
NSUPER = 8             # 8 superblocks of 512 rows
FP32 = mybir.dt.float32
FP16 = mybir.dt.float16
BF16 = mybir.dt.bfloat16

_PROG_CACHE = {}


def build_program():
    if "nc" in _PROG_CACHE:
        return _PROG_CACHE["nc"]
    nc = bacc.Bacc("TRN2", target_bir_lowering=False, debug=False)

    ident = nc.dram_tensor("ident", [128, 128], FP16, kind="ExternalInput")
    bones = nc.dram_tensor("bones", [128, 128], BF16, kind="ExternalInput")
    EFp = nc.dram_tensor("EFp", [128, NCHUNK * 2 * K], FP16, kind="ExternalInput")
    x_nat = nc.dram_tensor("x_nat", [N, DIM], FP16, kind="ExternalInput")
    Wk = nc.dram_tensor("Wk", [DIM, QC], FP16, kind="ExternalInput")
    Wv = nc.dram_tensor("Wv", [DIM, QC], FP16, kind="ExternalInput")
    WqT = nc.dram_tensor("WqT", [QC, DIM], FP16, kind="ExternalInput")
    WoB = nc.dram_tensor("WoB", [QC, DIM], FP16, kind="ExternalInput")
    bqp = nc.dram_tensor("bqp", [128, 4], FP16, kind="ExternalInput")
    r1k = nc.dram_tensor("r1k", [128, 4 * K], FP32, kind="ExternalInput")
    r1v = nc.dram_tensor("r1v", [128, 4 * K], FP32, kind="ExternalInput")
    xT = nc.dram_tensor("xT", [DIM, N], FP16, kind="ExternalInput")
    out_p = nc.dram_tensor("out_p", [N, DIM], FP16, kind="ExternalOutput")

    with tile.TileContext(nc) as tc, ExitStack() as ctx:
        singles = ctx.enter_context(tc.tile_pool(name="singles", bufs=1))

        # --- prologue: tiny DMAs first, then PE warmup under the x stream ---
        ident_t = singles.tile([128, 128], FP16)
        nc.sync.dma_start(ident_t[:], ident[:])
        bones_t = singles.tile([128, 128], BF16)
        nc.sync.dma_start(bones_t[:], bones[:])
        ef_t = singles.tile([128, NCHUNK, 2 * K], FP16)
        nc.sync.dma_start(ef_t[:], EFp[:].rearrange("p (i k) -> p i k", i=NCHUNK))

        wm_src = singles.tile([128, 512], FP16)
        nc.vector.memset(wm_src[:], 1.0)
        with tc.tile_pool(name="warm", bufs=1, space="PSUM") as warm_pool:
            wm_ps = warm_pool.tile([128, 512], FP32)
            for _ in range(14):
                nc.tensor.matmul(wm_ps[:], ident_t[:], wm_src[:],
                                 start=True, stop=True)

        # ---------------- Pass A: xEF = EF^T @ x  (fp32 psum) ----------------
        a2sb = ctx.enter_context(tc.tile_pool(name="a2sb", bufs=1))
        xe16 = a2sb.tile([64, DIM], FP16)
        xf16 = a2sb.tile([64, DIM], FP16)
        with tc.tile_pool(name="xef_ps", bufs=1, space="PSUM") as xef_pool:
            xef_ps = xef_pool.tile([128, DIM], FP32)
            with tc.tile_pool(name="xa", bufs=8) as xa_pool:
                for i in range(NCHUNK):
                    x_t = xa_pool.tile([128, DIM], FP16)
                    nc.sync.dma_start(x_t[:], x_nat[i * 128:(i + 1) * 128, :])
                    for f in (0, 512):
                        nc.tensor.matmul(xef_ps[:, f:f + 512], ef_t[:, i, :],
                                         x_t[:, f:f + 512],
                                         start=(i == 0), stop=(i == NCHUNK - 1))
            nc.vector.tensor_copy(xe16[:], xef_ps[0:64, :])
            nc.scalar.copy(xf16[:], xef_ps[64:128, :])

        # weight / constant DMAs (queued behind the x chunks; land during A2)
        wk_t = singles.tile([128, 8, QC], FP16)
        nc.sync.dma_start(wk_t[:], Wk[:].rearrange("(j p) c -> p j c", p=128))
        wv_t = singles.tile([128, 8, QC], FP16)
        nc.sync.dma_start(wv_t[:], Wv[:].rearrange("(j p) c -> p j c", p=128))
        r1k_t = singles.tile([128, 4, K], FP32)
        nc.sync.dma_start(r1k_t[:], r1k[:].rearrange("p (t k) -> p t k", t=4))
        r1v_t = singles.tile([128, 4, K], FP32)
        nc.sync.dma_start(r1v_t[:], r1v[:].rearrange("p (t k) -> p t k", t=4))
        bqp_t = singles.tile([128, 4], FP16)
        nc.sync.dma_start(bqp_t[:], bqp[:])
        wqt_t = singles.tile([128, 4, DIM], FP16)
        nc.sync.dma_start(wqt_t[:], WqT[:].rearrange("(t p) c -> p t c", p=128))
        wob_t = singles.tile([128, 4, DIM], FP16)
        nc.sync.dma_start(wob_t[:], WoB[:].rearrange("(t p) c -> p t c", p=128))

        # ---------------- Pass A2: klrT/vlrT -> kbd/vbd -> M, dcorrT, vw ----
        kbd = a2sb.tile([128, 4, 128], FP16)
        nc.vector.memset(kbd[:], 0.0)
        vbd = a2sb.tile([128, 4, 128], FP16)
        nc.vector.memset(vbd[:], 0.0)
        m_sb = a2sb.tile([128, 8, QC], FP16)
        vw_sb = a2sb.tile([128, 4, DIM], FP16)
        expbias = a2sb.tile([128, 4], FP32)

        with tc.tile_pool(name="a2ps", bufs=2, space="PSUM") as a2ps:
            # transpose xE/xF: [64, 1024] -> 8 chunks of [128, 64] each
            xet_sb = a2sb.tile([128, 8, K], FP16)
            xft_sb = a2sb.tile([128, 8, K], FP16)
            for (src, dst) in ((xe16, xet_sb), (xf16, xft_sb)):
                tp = a2ps.tile([128, 8, K], FP16, tag="xt_ps")
                for j in range(8):
                    nc.tensor.transpose(
                        tp[:, j, :], src[:, j * 128:(j + 1) * 128],
                        ident_t[0:64, 0:64],
                    )
                nc.vector.tensor_copy(dst[:], tp[:])

            # klrT/vlrT tiles [c_pair, k] directly; + rank-1 bias on the copy
            for (w_t, xt, r1, dst) in (
                (wk_t, xet_sb, r1k_t, kbd),
                (wv_t, xft_sb, r1v_t, vbd),
            ):
                for t in range(4):
                    lp = a2ps.tile([128, K], FP32, tag="lr_ps")
                    for j in range(8):
                        nc.tensor.matmul(
                            lp[:], w_t[:, j, t * 128:(t + 1) * 128], xt[:, j, :],
                            start=(j == 0), stop=(j == 7),
                        )
                    nc.vector.tensor_add(out=dst[0:64, t, 0:64],
                                         in0=lp[0:64, :], in1=r1[0:64, t, :])
                    nc.vector.tensor_add(out=dst[64:128, t, 64:128],
                                         in0=lp[64:128, :], in1=r1[64:128, t, :])

            # M tiles: m_sb[p, j, hk] = M[j*128+p, hk] = (Wq klr^T)[d, hk]
            for j in range(8):
                m_ps = a2ps.tile([128, QC], FP32, tag="big")
                for t in range(4):
                    nc.tensor.matmul(
                        m_ps[:, t * 128:(t + 1) * 128],
                        wqt_t[:, t, j * 128:(j + 1) * 128],
                        kbd[:, t, :],
                        start=True, stop=True,
                    )
                if j % 2 == 0:
                    nc.vector.tensor_copy(m_sb[:, j, :], m_ps[:])
                else:
                    nc.scalar.copy(m_sb[:, j, :], m_ps[:])

            # dcorrT[hk] = bq . klr[hk-row]; exp bias = 0.125*dcorrT - 80
            dc_big = a2ps.tile([128, 512], FP32, tag="big")
            dc_ps = dc_big[:, 0:4]
            for t in range(4):
                nc.tensor.matmul(dc_ps[:, t:t + 1], kbd[:, t, :],
                                 bqp_t[:, t:t + 1], start=True, stop=True)
            nc.scalar.activation(out=expbias[:], in_=dc_ps[:],
                                 func=mybir.ActivationFunctionType.Copy,
                                 bias=-80.0, scale=0.125)

            # vw: pair-stacked (vlr_h @ Wout_h) in fp16
            for t in range(4):
                for f in (0, 512):
                    vw_ps = a2ps.tile([128, 512], FP32, tag="big")
                    nc.tensor.matmul(vw_ps[:], vbd[:, t, :],
                                     wob_t[:, t, f:f + 512],
                                     start=True, stop=True)
                    if f == 0:
                        nc.vector.tensor_copy(vw_sb[:, t, f:f + 512], vw_ps[:])
                    else:
                        nc.scalar.copy(vw_sb[:, t, f:f + 512], vw_ps[:])

        # ---------------- Pass B: dotsT -> softmax -> out --------------------
        xt_pool = ctx.enter_context(tc.tile_pool(name="xt", bufs=3))
        exp_pool = ctx.enter_context(tc.tile_pool(name="expp", bufs=5))
        attn_pool = ctx.enter_context(tc.tile_pool(name="attn", bufs=2))
        osb_pool = ctx.enter_context(tc.tile_pool(name="osb", bufs=3))
        dots_pool = ctx.enter_context(tc.tile_pool(name="dots", bufs=2, space="PSUM"))
        sums_pool = ctx.enter_context(tc.tile_pool(name="sums", bufs=2, space="PSUM"))
        out_ps_pool = ctx.enter_context(tc.tile_pool(name="outps", bufs=2, space="PSUM"))

        xts_tiles = [None] * NSUPER
        attn_tiles = [None] * NSUPER

        def front_half(s):
            """dotsT -> exp -> sums -> attnT for superblock s."""
            xts = xt_pool.tile([128, 8, 512], FP16)
            nc.sync.dma_start(
                xts[:],
                xT[:, s * 512:(s + 1) * 512].rearrange("(j p) n -> p j n", p=128),
            )
            xts_tiles[s] = xts
            attnT = attn_pool.tile([128, 4, 512], FP16)
            attn_tiles[s] = attnT
            for t in range(4):
                dots_ps = dots_pool.tile([128, 512], FP32)
                for j in range(8):
                    nc.tensor.matmul(
                        dots_ps[:], m_sb[:, j, t * 128:(t + 1) * 128],
                        xts[:, j, :],
                        start=(j == 0), stop=(j == 7),
                    )
                expT = exp_pool.tile([128, 512], BF16)
                nc.scalar.activation(
                    out=expT[:], in_=dots_ps[:],
                    func=mybir.ActivationFunctionType.Exp,
                    scale=0.125, bias=expbias[:, t:t + 1],
                )
                sums_ps = sums_pool.tile([128, 512], FP32)
                nc.tensor.matmul(sums_ps[:], bones_t[:], expT[:],
                                 start=True, stop=True)
                rec = exp_pool.tile([128, 512], FP32, tag="rec")
                nc.vector.reciprocal_approx_fast(out=rec[:], in_=sums_ps[:])
                nc.vector.tensor_mul(out=attnT[:, t, :], in0=expT[:], in1=rec[:])

        def back_half(s):
            """attnT @ vw -> out rows for superblock s."""
            attnT = attn_tiles[s]
            for q in range(4):
                out_ps = out_ps_pool.tile([128, DIM], FP32)
                for f in (0, 512):
                    for t in range(4):
                        nc.tensor.matmul(
                            out_ps[:, f:f + 512],
                            attnT[:, t, q * 128:(q + 1) * 128],
                            vw_sb[:, t, f:f + 512],
                            start=(t == 0), stop=(t == 3),
                        )
                out_sb = osb_pool.tile([128, DIM], FP16)
                nc.vector.tensor_copy(out_sb[:, 0:512], out_ps[:, 0:512])
                nc.scalar.copy(out_sb[:, 512:1024], out_ps[:, 512:1024])
                i = s * 4 + q
                nc.gpsimd.dma_start(out_p[i * 128:(i + 1) * 128, :], out_sb[:])

        # software-pipeline by one superblock so the PE never waits on softmax
        front_half(0)
        for s in range(1, NSUPER):
            front_half(s)
            back_half(s - 1)
        back_half(NSUPER - 1)

    nc.finalize()
    _PROG_CACHE["nc"] = nc
    return nc


def shard_inputs(x, E, F, W_qkv, b_qkv, W_out, b_out):
    """Host-side prep: slice / transpose / cast per core."""
    x = np.asarray(x, dtype=np.float32)
    E = np.asarray(E, dtype=np.float32)
    F = np.asarray(F, dtype=np.float32)
    W_qkv = np.asarray(W_qkv, dtype=np.float32)
    b_qkv = np.asarray(b_qkv, dtype=np.float32)
    W_out = np.asarray(W_out, dtype=np.float32)

    sE = E.sum(0).astype(np.float32)   # [K]
    sF = F.sum(0).astype(np.float32)
    # EF pre-chunked: EFp[p, i*2K + k] = EF[i*128 + p, k]
    EF = np.concatenate([E, F], axis=1).astype(np.float16)       # [N, 2K]
    EFp = np.ascontiguousarray(
        EF.reshape(NCHUNK, 128, 2 * K).transpose(1, 0, 2).reshape(128, -1))

    ident = np.eye(128, dtype=np.float16)
    bones = np.zeros((128, 128), np.float32)
    bones[:64, :64] = 1.0
    bones[64:, 64:] = 1.0
    import ml_dtypes
    bones = bones.astype(ml_dtypes.bfloat16)

    in_maps = []
    xb_cache = {}
    for c in range(NCORES):
        b, hg = c // 2, c % 2
        hs = NH * hg
        if b not in xb_cache:
            xb16 = np.ascontiguousarray(x[:, b, :]).astype(np.float16)
            xT16 = np.ascontiguousarray(xb16.T)
            xb_cache[b] = (xb16, xT16)
        xb16, xT16 = xb_cache[b]

        qcols = slice(hs * DH, (hs + NH) * DH)
        kcols = slice(DIM + hs * DH, DIM + (hs + NH) * DH)
        vcols = slice(2 * DIM + hs * DH, 2 * DIM + (hs + NH) * DH)

        bq = b_qkv[qcols]                      # [512]
        bqp16 = np.ascontiguousarray(
            bq.reshape(4, 128).T).astype(np.float16)             # [128, 4]
        bk = b_qkv[kcols]
        bv = b_qkv[vcols]
        # r1kT[p, t*K + k] = bk[t*128 + p] * sE[k]
        r1kT = np.ascontiguousarray(
            (bk.reshape(4, 128)[:, :, None] * sE[None, None, :])
            .transpose(1, 0, 2).reshape(128, 4 * K))
        r1vT = np.ascontiguousarray(
            (bv.reshape(4, 128)[:, :, None] * sF[None, None, :])
            .transpose(1, 0, 2).reshape(128, 4 * K))

        in_maps.append({
            "ident": ident,
            "bones": bones,
            "EFp": EFp,
            "x_nat": xb16,
            "Wk": W_qkv[:, kcols].astype(np.float16),
            "Wv": W_qkv[:, vcols].astype(np.float16),
            "WqT": np.ascontiguousarray(W_qkv[:, qcols].T).astype(np.float16),
            "WoB": W_out[hs * DH:(hs + NH) * DH, :].astype(np.float16),
            "bqp": bqp16,
            "r1k": r1kT.astype(np.float32),
            "r1v": r1vT.astype(np.float32),
            "xT": xT16,
        })
    return in_maps


def kernel_impl(inputs, trace=False, **run_kwargs):
    nc = build_program()
    in_maps = shard_inputs(
        inputs["x"], inputs["E"], inputs["F"], inputs["W_qkv"],
        inputs["b_qkv"], inputs["W_out"], inputs["b_out"],
    )
    res = run_bass_kernel_spmd(nc, in_maps, list(range(NCORES)),
                               trace=trace, **run_kwargs)
    b_out = np.asarray(inputs["b_out"], dtype=np.float32)
    out = np.empty((N, B, DIM), np.float32)
    for b in range(B):
        out[:, b, :] = (res.results[2 * b]["out_p"].astype(np.float32)
                        + res.results[2 * b + 1]["out_p"].astype(np.float32)
                        + b_out)
    return out, res


def kernel(**inputs):
    out, _ = kernel_impl(inputs)
    return out


# revision 11
# speedup vs baseline: 1.4601x; 1.0197x over previous
"""Linformer attention Trainium2 kernel (8-core SPMD, batch x head-group sharded).

Sharding: core c handles batch b = c//2 and heads [8*(c%2), 8*(c%2)+8).
Each core computes a partial output (contribution of its 8 heads to its batch);
the host sums the two partials per batch and adds b_out.

Math per core (b, heads hs..hs+8), exploiting the Linformer low-rank structure:
  xEF  = EF^T @ x_b                  (128 x 1024 fp32 psum; E rows 0:64, F 64:128)
  klrT = Wk-chunks^T @ xET-chunks    ([c, k] layout directly; + rank-1 bias on copy)
  kbd  = pair-stacked block-diag klrT (fp16)        vbd likewise from vlrT
  M    = WqT-tiles^T @ kbd           (m_sb [d, hk], fp16)
  dcorrT = kbd^T @ bq                (per-hk dots bias -> folded into exp bias)
  vw   = vbd^T @ WoB-tiles           (vw_sb [hk, e], fp16)
  per 512-row superblock (transposed pass B; no attn transposes needed):
    dotsT_t = sum_j m_sb[:,j,t]^T @ xT-tiles       ([hk, r] in psum)
    expT_t  = Exp(0.125*dotsT_t + bias_t)          (ACT, per-partition bias, bf16)
    sums_t  = blockones^T @ expT_t                 (per-head partition sums, bcast)
    attnT_t = expT_t / sums_t                      (DVE divide, fp16)
    out_q  += attnT_t-tiles^T @ vw_sb              (fp16 matmul, fp32 accum)
The q/k chain stays fp16-operand (fp32 accumulation) end to end; softmax uses a
constant shift (exp args stay in bf16 range; see baseline analysis).
"""

import sys

import numpy as np

try:
    import concourse.bass as bass  # noqa: F401
except ImportError:
    sys.path.insert(0, "/opt/trn_rl_repo")

from contextlib import ExitStack

import concourse.bass as bass
import concourse.tile as tile
from concourse import bacc, mybir
from concourse.bass_utils import run_bass_kernel_spmd

N, B, DIM, H, K, DH = 4096, 4, 1024, 16, 64, 64
NH = 8           # heads per core
QC = NH * DH     # 512, per-core q/k/v column span
NCORES = 8
NCHUNK = N // 128      # 32 row chunks
NSUPER = 8             # 8 superblocks of 512 rows
FP32 = mybir.dt.float32
FP16 = mybir.dt.float16
BF16 = mybir.dt.bfloat16

_PROG_CACHE = {}


def build_program():
    if "nc" in _PROG_CACHE:
        return _PROG_CACHE["nc"]
    nc = bacc.Bacc("TRN2", target_bir_lowering=False, debug=False)

    ident = nc.dram_tensor("ident", [128, 128], FP16, kind="ExternalInput")
    bones = nc.dram_tensor("bones", [128, 128], BF16, kind="ExternalInput")
    EFp = nc.dram_tensor("EFp", [128, NCHUNK * 2 * K], FP16, kind="ExternalInput")
    x_nat = nc.dram_tensor("x_nat", [N, DIM], FP16, kind="ExternalInput")
    Wk = nc.dram_tensor("Wk", [DIM, QC], FP16, kind="ExternalInput")
    Wv = nc.dram_tensor("Wv", [DIM, QC], FP16, kind="ExternalInput")
    WqT = nc.dram_tensor("WqT", [QC, DIM], FP16, kind="ExternalInput")
    WoB = nc.dram_tensor("WoB", [QC, DIM], FP16, kind="ExternalInput")
    bqp = nc.dram_tensor("bqp", [128, 4], FP16, kind="ExternalInput")
    r1k = nc.dram_tensor("r1k", [128, 4 * K], FP32, kind="ExternalInput")
    r1v = nc.dram_tensor("r1v", [128, 4 * K], FP32, kind="ExternalInput")
    xT = nc.dram_tensor("xT", [DIM, N], FP16, kind="ExternalInput")
    out_p = nc.dram_tensor("out_p", [N, DIM], FP16, kind="ExternalOutput")

    with tile.TileContext(nc) as tc, ExitStack() as ctx:
        singles = ctx.enter_context(tc.tile_pool(name="singles", bufs=1))

        # --- prologue ---
        # small/early tensors go on the scalar DMA queue so they land during
        # pass A without delaying the x stream on the sync queue
        ident_t = singles.tile([128, 128], FP16)
        nc.scalar.dma_start(ident_t[:], ident[:])
        bones_t = singles.tile([128, 128], BF16)
        nc.scalar.dma_start(bones_t[:], bones[:])
        bqp_t = singles.tile([128, 4], FP16)
        nc.scalar.dma_start(bqp_t[:], bqp[:])
        r1k_t = singles.tile([128, 4, K], FP32)
        nc.scalar.dma_start(r1k_t[:], r1k[:].rearrange("p (t k) -> p t k", t=4))
        r1v_t = singles.tile([128, 4, K], FP32)
        nc.scalar.dma_start(r1v_t[:], r1v[:].rearrange("p (t k) -> p t k", t=4))
        wk_t = singles.tile([128, 8, QC], FP16)
        nc.scalar.dma_start(wk_t[:], Wk[:].rearrange("(j p) c -> p j c", p=128))
        wv_t = singles.tile([128, 8, QC], FP16)
        nc.scalar.dma_start(wv_t[:], Wv[:].rearrange("(j p) c -> p j c", p=128))

        ef_t = singles.tile([128, NCHUNK, 2 * K], FP16)
        nc.sync.dma_start(ef_t[:], EFp[:].rearrange("p (i k) -> p i k", i=NCHUNK))

        # PE warmup: depends only on the DVE memset, so the HAM clock-gate
        # lifts (~3.4us of sustained activity) before pass A's matmuls arrive
        wm_src = singles.tile([128, 512], FP16)
        nc.vector.memset(wm_src[:], 1.0)
        warm_cm = tc.tile_pool(name="warm", bufs=1, space="PSUM")
        warm_pool = warm_cm.__enter__()
        wm_ps = warm_pool.tile([128, 512], FP32)

        def filler_mms(n):
            for _ in range(n):
                nc.tensor.matmul(wm_ps[:], wm_src[:, 0:128], wm_src[:],
                                 start=True, stop=True)

        filler_mms(14)

        # ---------------- Pass A: xEF = EF^T @ x  (fp32 psum) ----------------
        a2sb = ctx.enter_context(tc.tile_pool(name="a2sb", bufs=1))
        xe16 = a2sb.tile([64, DIM], FP16)
        xf16 = a2sb.tile([64, DIM], FP16)
        with tc.tile_pool(name="xef_ps", bufs=1, space="PSUM") as xef_pool:
            xef_ps = xef_pool.tile([128, DIM], FP32)
            with tc.tile_pool(name="xa", bufs=8) as xa_pool:
                for i in range(NCHUNK):
                    x_t = xa_pool.tile([128, DIM], FP16)
                    nc.sync.dma_start(x_t[:], x_nat[i * 128:(i + 1) * 128, :])
                    for f in (0, 512):
                        nc.tensor.matmul(xef_ps[:, f:f + 512], ef_t[:, i, :],
                                         x_t[:, f:f + 512],
                                         start=(i == 0), stop=(i == NCHUNK - 1))
            # keep the PE busy through the copy/transpose gap so the HAM
            # clock-gate doesn't re-throttle right before A2
            filler_mms(6)
            nc.vector.tensor_copy(xe16[:], xef_ps[0:64, :])
            nc.scalar.copy(xf16[:], xef_ps[64:128, :])

        # big A2 weights: behind the x chunks on the sync queue
        wqt_t = singles.tile([128, 4, DIM], FP16)
        nc.sync.dma_start(wqt_t[:], WqT[:].rearrange("(t p) c -> p t c", p=128))
        wob_t = singles.tile([128, 4, DIM], FP16)
        nc.sync.dma_start(wob_t[:], WoB[:].rearrange("(t p) c -> p t c", p=128))

        # ---------------- Pass A2: klrT/vlrT -> kbd/vbd -> M, dcorrT, vw ----
        kbd = a2sb.tile([128, 4, 128], FP16)
        nc.vector.memset(kbd[:], 0.0)
        vbd = a2sb.tile([128, 4, 128], FP16)
        nc.vector.memset(vbd[:], 0.0)
        m_sb = a2sb.tile([128, 8, QC], FP16)
        vw_sb = a2sb.tile([128, 4, DIM], FP16)
        expbias = a2sb.tile([128, 4], FP32)

        with tc.tile_pool(name="a2ps", bufs=2, space="PSUM") as a2ps:
            # transpose xE/xF: [64, 1024] -> 8 chunks of [128, 64] each
            xet_sb = a2sb.tile([128, 8, K], FP16)
            xft_sb = a2sb.tile([128, 8, K], FP16)
            for (src, dst) in ((xe16, xet_sb), (xf16, xft_sb)):
                tp = a2ps.tile([128, 8, K], FP16, tag="xt_ps")
                for j in range(8):
                    nc.tensor.transpose(
                        tp[:, j, :], src[:, j * 128:(j + 1) * 128],
                        ident_t[0:64, 0:64],
                    )
                nc.vector.tensor_copy(dst[:], tp[:])

            # klrT/vlrT tiles [c_pair, k] directly; + rank-1 bias on the copy
            for (w_t, xt, r1, dst) in (
                (wk_t, xet_sb, r1k_t, kbd),
                (wv_t, xft_sb, r1v_t, vbd),
            ):
                for t in range(4):
                    lp = a2ps.tile([128, K], FP32, tag="lr_ps")
                    for j in range(8):
                        nc.tensor.matmul(
                            lp[:], w_t[:, j, t * 128:(t + 1) * 128], xt[:, j, :],
                            start=(j == 0), stop=(j == 7),
                        )
                    nc.vector.tensor_add(out=dst[0:64, t, 0:64],
                                         in0=lp[0:64, :], in1=r1[0:64, t, :])
                    nc.vector.tensor_add(out=dst[64:128, t, 64:128],
                                         in0=lp[64:128, :], in1=r1[64:128, t, :])

            # bridge the wait for the wqt DMA (keeps the clock-gate warm)
            filler_mms(6)

            # M tiles: m_sb[p, j, hk] = M[j*128+p, hk] = (Wq klr^T)[d, hk]
            for j in range(8):
                m_ps = a2ps.tile([128, QC], FP32, tag="big")
                for t in range(4):
                    nc.tensor.matmul(
                        m_ps[:, t * 128:(t + 1) * 128],
                        wqt_t[:, t, j * 128:(j + 1) * 128],
                        kbd[:, t, :],
                        start=True, stop=True,
                    )
                if j % 2 == 0:
                    nc.vector.tensor_copy(m_sb[:, j, :], m_ps[:])
                else:
                    nc.scalar.copy(m_sb[:, j, :], m_ps[:])

            # dcorrT[hk] = bq . klr[hk-row]; exp bias = 0.125*dcorrT - 80
            dc_big = a2ps.tile([128, 512], FP32, tag="big")
            dc_ps = dc_big[:, 0:4]
            for t in range(4):
                nc.tensor.matmul(dc_ps[:, t:t + 1], kbd[:, t, :],
                                 bqp_t[:, t:t + 1], start=True, stop=True)
            nc.scalar.activation(out=expbias[:], in_=dc_ps[:],
                                 func=mybir.ActivationFunctionType.Copy,
                                 bias=-80.0, scale=0.125)

            # vw: pair-stacked (vlr_h @ Wout_h) in fp16
            for t in range(4):
                for f in (0, 512):
                    vw_ps = a2ps.tile([128, 512], FP32, tag="big")
                    nc.tensor.matmul(vw_ps[:], vbd[:, t, :],
                                     wob_t[:, t, f:f + 512],
                                     start=True, stop=True)
                    if f == 0:
                        nc.vector.tensor_copy(vw_sb[:, t, f:f + 512], vw_ps[:])
                    else:
                        nc.scalar.copy(vw_sb[:, t, f:f + 512], vw_ps[:])

        warm_cm.__exit__(None, None, None)

        # ---------------- Pass B: dotsT -> softmax -> out --------------------
        xt_pool = ctx.enter_context(tc.tile_pool(name="xt", bufs=3))
        exp_pool = ctx.enter_context(tc.tile_pool(name="expp", bufs=5))
        attn_pool = ctx.enter_context(tc.tile_pool(name="attn", bufs=2))
        osb_pool = ctx.enter_context(tc.tile_pool(name="osb", bufs=3))
        dots_pool = ctx.enter_context(tc.tile_pool(name="dots", bufs=2, space="PSUM"))
        sums_pool = ctx.enter_context(tc.tile_pool(name="sums", bufs=2, space="PSUM"))
        out_ps_pool = ctx.enter_context(tc.tile_pool(name="outps", bufs=2, space="PSUM"))

        xts_tiles = [None] * NSUPER
        attn_tiles = [None] * NSUPER

        def front_half(s):
            """dotsT -> exp -> sums -> attnT for superblock s."""
            xts = xt_pool.tile([128, 8, 512], FP16)
            nc.sync.dma_start(
                xts[:],
                xT[:, s * 512:(s + 1) * 512].rearrange("(j p) n -> p j n", p=128),
            )
            xts_tiles[s] = xts
            attnT = attn_pool.tile([128, 4, 512], FP16)
            attn_tiles[s] = attnT
            for t in range(4):
                dots_ps = dots_pool.tile([128, 512], FP32)
                for j in range(8):
                    nc.tensor.matmul(
                        dots_ps[:], m_sb[:, j, t * 128:(t + 1) * 128],
                        xts[:, j, :],
                        start=(j == 0), stop=(j == 7),
                    )
                expT = exp_pool.tile([128, 512], BF16)
                nc.scalar.activation(
                    out=expT[:], in_=dots_ps[:],
                    func=mybir.ActivationFunctionType.Exp,
                    scale=0.125, bias=expbias[:, t:t + 1],
                )
                sums_ps = sums_pool.tile([128, 512], FP32)
                nc.tensor.matmul(sums_ps[:], bones_t[:], expT[:],
                                 start=True, stop=True)
                rec = exp_pool.tile([128, 512], FP32, tag="rec")
                nc.vector.reciprocal_approx_fast(out=rec[:], in_=sums_ps[:])
                nc.vector.tensor_mul(out=attnT[:, t, :], in0=expT[:], in1=rec[:])

        def back_half(s):
            """attnT @ vw -> out rows for superblock s."""
            attnT = attn_tiles[s]
            for q in range(4):
                out_ps = out_ps_pool.tile([128, DIM], FP32)
                for f in (0, 512):
                    for t in range(4):
                        nc.tensor.matmul(
                            out_ps[:, f:f + 512],
                            attnT[:, t, q * 128:(q + 1) * 128],
                            vw_sb[:, t, f:f + 512],
                            start=(t == 0), stop=(t == 3),
                        )
                out_sb = osb_pool.tile([128, DIM], FP16)
                nc.vector.tensor_copy(out_sb[:, 0:512], out_ps[:, 0:512])
                nc.scalar.copy(out_sb[:, 512:1024], out_ps[:, 512:1024])
                i = s * 4 + q
                nc.gpsimd.dma_start(out_p[i * 128:(i + 1) * 128, :], out_sb[:])

        # software-pipeline by one superblock so the PE never waits on softmax
        front_half(0)
        for s in range(1, NSUPER):
            front_half(s)
            back_half(s - 1)
        back_half(NSUPER - 1)

    nc.finalize()
    _PROG_CACHE["nc"] = nc
    return nc


def shard_inputs(x, E, F, W_qkv, b_qkv, W_out, b_out):
    """Host-side prep: slice / transpose / cast per core."""
    x = np.asarray(x, dtype=np.float32)
    E = np.asarray(E, dtype=np.float32)
    F = np.asarray(F, dtype=np.float32)
    W_qkv = np.asarray(W_qkv, dtype=np.float32)
    b_qkv = np.asarray(b_qkv, dtype=np.float32)
    W_out = np.asarray(W_out, dtype=np.float32)

    sE = E.sum(0).astype(np.float32)   # [K]
    sF = F.sum(0).astype(np.float32)
    # EF pre-chunked: EFp[p, i*2K + k] = EF[i*128 + p, k]
    EF = np.concatenate([E, F], axis=1).astype(np.float16)       # [N, 2K]
    EFp = np.ascontiguousarray(
        EF.reshape(NCHUNK, 128, 2 * K).transpose(1, 0, 2).reshape(128, -1))

    ident = np.eye(128, dtype=np.float16)
    bones = np.zeros((128, 128), np.float32)
    bones[:64, :64] = 1.0
    bones[64:, 64:] = 1.0
    import ml_dtypes
    bones = bones.astype(ml_dtypes.bfloat16)

    in_maps = []
    xb_cache = {}
    for c in range(NCORES):
        b, hg = c // 2, c % 2
        hs = NH * hg
        if b not in xb_cache:
            xb16 = np.ascontiguousarray(x[:, b, :]).astype(np.float16)
            xT16 = np.ascontiguousarray(xb16.T)
            xb_cache[b] = (xb16, xT16)
        xb16, xT16 = xb_cache[b]

        qcols = slice(hs * DH, (hs + NH) * DH)
        kcols = slice(DIM + hs * DH, DIM + (hs + NH) * DH)
        vcols = slice(2 * DIM + hs * DH, 2 * DIM + (hs + NH) * DH)

        bq = b_qkv[qcols]                      # [512]
        bqp16 = np.ascontiguousarray(
            bq.reshape(4, 128).T).astype(np.float16)             # [128, 4]
        bk = b_qkv[kcols]
        bv = b_qkv[vcols]
        # r1kT[p, t*K + k] = bk[t*128 + p] * sE[k]
        r1kT = np.ascontiguousarray(
            (bk.reshape(4, 128)[:, :, None] * sE[None, None, :])
            .transpose(1, 0, 2).reshape(128, 4 * K))
        r1vT = np.ascontiguousarray(
            (bv.reshape(4, 128)[:, :, None] * sF[None, None, :])
            .transpose(1, 0, 2).reshape(128, 4 * K))

        in_maps.append({
            "ident": ident,
            "bones": bones,
            "EFp": EFp,
            "x_nat": xb16,
            "Wk": W_qkv[:, kcols].astype(np.float16),
            "Wv": W_qkv[:, vcols].astype(np.float16),
            "WqT": np.ascontiguousarray(W_qkv[:, qcols].T).astype(np.float16),
            "WoB": W_out[hs * DH:(hs + NH) * DH, :].astype(np.float16),
            "bqp": bqp16,
            "r1k": r1kT.astype(np.float32),
            "r1v": r1vT.astype(np.float32),
            "xT": xT16,
        })
    return in_maps


def kernel_impl(inputs, trace=False, **run_kwargs):
    nc = build_program()
    in_maps = shard_inputs(
        inputs["x"], inputs["E"], inputs["F"], inputs["W_qkv"],
        inputs["b_qkv"], inputs["W_out"], inputs["b_out"],
    )
    res = run_bass_kernel_spmd(nc, in_maps, list(range(NCORES)),
                               trace=trace, **run_kwargs)
    b_out = np.asarray(inputs["b_out"], dtype=np.float32)
    out = np.empty((N, B, DIM), np.float32)
    for b in range(B):
        out[:, b, :] = (res.results[2 * b]["out_p"].astype(np.float32)
                        + res.results[2 * b + 1]["out_p"].astype(np.float32)
                        + b_out)
    return out, res


def kernel(**inputs):
    out, _ = kernel_impl(inputs)
    return out


# revision 15
# speedup vs baseline: 1.4864x; 1.0180x over previous
"""Linformer attention Trainium2 kernel (8-core SPMD, batch x head-group sharded).

Sharding: core c handles batch b = c//2 and heads [8*(c%2), 8*(c%2)+8).
Each core computes a partial output (contribution of its 8 heads to its batch);
the host sums the two partials per batch and adds b_out.

Math per core (b, heads hs..hs+8), exploiting the Linformer low-rank structure:
  xEF  = EF^T @ x_b                  (128 x 1024 fp32 psum; E rows 0:64, F 64:128)
  klrT = Wk-chunks^T @ xET-chunks    ([c, k] layout directly; + rank-1 bias on copy)
  kbd  = pair-stacked block-diag klrT (fp16)        vbd likewise from vlrT
  M    = WqT-tiles^T @ kbd           (m_sb [d, hk], fp16)
  dcorrT = kbd^T @ bq                (per-hk dots bias -> folded into exp bias)
  vw   = vbd^T @ WoB-tiles           (vw_sb [hk, e], fp16)
  per 512-row superblock (transposed pass B; no attn transposes needed):
    dotsT_t = sum_j m_sb[:,j,t]^T @ xT-tiles       ([hk, r] in psum)
    expT_t  = Exp(0.125*dotsT_t + bias_t)          (ACT, per-partition bias, bf16)
    sums_t  = blockones^T @ expT_t                 (per-head partition sums, bcast)
    attnT_t = expT_t / sums_t                      (DVE divide, fp16)
    out_q  += attnT_t-tiles^T @ vw_sb              (fp16 matmul, fp32 accum)
The q/k chain stays fp16-operand (fp32 accumulation) end to end; softmax uses a
constant shift (exp args stay in bf16 range; see baseline analysis).
"""

import sys

import numpy as np

try:
    import concourse.bass as bass  # noqa: F401
except ImportError:
    sys.path.insert(0, "/opt/trn_rl_repo")

from contextlib import ExitStack

import concourse.bass as bass
import concourse.tile as tile
from concourse import bacc, mybir
from concourse.bass_utils import run_bass_kernel_spmd

N, B, DIM, H, K, DH = 4096, 4, 1024, 16, 64, 64
NH = 8           # heads per core
QC = NH * DH     # 512, per-core q/k/v column span
NCORES = 8
NCHUNK = N // 128      # 32 row chunks
NSUPER = 8             # 8 superblocks of 512 rows
FP32 = mybir.dt.float32
FP16 = mybir.dt.float16
BF16 = mybir.dt.bfloat16

_PROG_CACHE = {}


def build_program():
    if "nc" in _PROG_CACHE:
        return _PROG_CACHE["nc"]
    nc = bacc.Bacc("TRN2", target_bir_lowering=False, debug=False)

    ident = nc.dram_tensor("ident", [128, 128], FP16, kind="ExternalInput")
    bones = nc.dram_tensor("bones", [128, 128], BF16, kind="ExternalInput")
    EFp = nc.dram_tensor("EFp", [128, NCHUNK * 2 * K], FP16, kind="ExternalInput")
    x_nat = nc.dram_tensor("x_nat", [N, DIM], FP16, kind="ExternalInput")
    Wk = nc.dram_tensor("Wk", [DIM, QC], FP16, kind="ExternalInput")
    Wv = nc.dram_tensor("Wv", [DIM, QC], FP16, kind="ExternalInput")
    WqT = nc.dram_tensor("WqT", [QC, DIM], FP16, kind="ExternalInput")
    WoB = nc.dram_tensor("WoB", [QC, DIM], FP16, kind="ExternalInput")
    bqp = nc.dram_tensor("bqp", [128, 4], FP16, kind="ExternalInput")
    r1k = nc.dram_tensor("r1k", [128, 4 * K], FP32, kind="ExternalInput")
    r1v = nc.dram_tensor("r1v", [128, 4 * K], FP32, kind="ExternalInput")
    xT = nc.dram_tensor("xT", [DIM, N], FP16, kind="ExternalInput")
    out_p = nc.dram_tensor("out_p", [N, DIM], FP16, kind="ExternalOutput")

    with tile.TileContext(nc) as tc, ExitStack() as ctx:
        singles = ctx.enter_context(tc.tile_pool(name="singles", bufs=1))

        # --- prologue ---
        # small/early tensors go on the scalar DMA queue so they land during
        # pass A without delaying the x stream on the sync queue
        ident_t = singles.tile([128, 128], FP16)
        nc.scalar.dma_start(ident_t[:], ident[:])
        bones_t = singles.tile([128, 128], BF16)
        nc.scalar.dma_start(bones_t[:], bones[:])
        bqp_t = singles.tile([128, 4], FP16)
        nc.scalar.dma_start(bqp_t[:], bqp[:])
        r1k_t = singles.tile([128, 4, K], FP32)
        nc.scalar.dma_start(r1k_t[:], r1k[:].rearrange("p (t k) -> p t k", t=4))
        r1v_t = singles.tile([128, 4, K], FP32)
        nc.scalar.dma_start(r1v_t[:], r1v[:].rearrange("p (t k) -> p t k", t=4))
        wk_t = singles.tile([128, 8, QC], FP16)
        wv_t = singles.tile([128, 8, QC], FP16)

        ef_t = singles.tile([128, NCHUNK, 2 * K], FP16)
        nc.sync.dma_start(ef_t[:], EFp[:].rearrange("p (i k) -> p i k", i=NCHUNK))

        # PE warmup: depends only on the DVE memset, so the HAM clock-gate
        # lifts (~3.4us of sustained activity) before pass A's matmuls arrive
        wm_src = singles.tile([128, 512], FP16)
        nc.vector.memset(wm_src[:], 1.0)
        warm_cm = tc.tile_pool(name="warm", bufs=1, space="PSUM")
        warm_pool = warm_cm.__enter__()
        wm_ps = warm_pool.tile([128, 512], FP32)

        def filler_mms(n):
            for _ in range(n):
                nc.tensor.matmul(wm_ps[:], wm_src[:, 0:128], wm_src[:],
                                 start=True, stop=True)

        filler_mms(14)

        # ---------------- Pass A: xEF = EF^T @ x  (fp32 psum) ----------------
        a2sb = ctx.enter_context(tc.tile_pool(name="a2sb", bufs=1))
        xe16 = a2sb.tile([64, DIM], FP16)
        xf16 = a2sb.tile([64, DIM], FP16)
        with tc.tile_pool(name="xef_ps", bufs=1, space="PSUM") as xef_pool:
            xef_ps = xef_pool.tile([128, DIM], FP32)
            with tc.tile_pool(name="xa", bufs=8) as xa_pool:
                for i in range(NCHUNK):
                    if i == 20:
                        # k/v weights slot in behind most of the x stream;
                        # they're first needed a few us after pass A ends
                        nc.sync.dma_start(
                            wk_t[:], Wk[:].rearrange("(j p) c -> p j c", p=128))
                        nc.sync.dma_start(
                            wv_t[:], Wv[:].rearrange("(j p) c -> p j c", p=128))
                    x_t = xa_pool.tile([128, DIM], FP16)
                    nc.sync.dma_start(x_t[:], x_nat[i * 128:(i + 1) * 128, :])
                    for f in (0, 512):
                        nc.tensor.matmul(xef_ps[:, f:f + 512], ef_t[:, i, :],
                                         x_t[:, f:f + 512],
                                         start=(i == 0), stop=(i == NCHUNK - 1))
            # keep the PE busy through the copy/transpose gap so the HAM
            # clock-gate doesn't re-throttle right before A2; evacuate
            # per-128-column chunks so the transposes start immediately
            filler_mms(4)
            for j in range(8):
                cs = slice(j * 128, (j + 1) * 128)
                nc.vector.tensor_copy(xe16[:, cs], xef_ps[0:64, cs])
                nc.scalar.copy(xf16[:, cs], xef_ps[64:128, cs])

        # big A2 weights + first xT superblock: behind x on the sync queue
        wqt_t = singles.tile([128, 4, DIM], FP16)
        nc.sync.dma_start(wqt_t[:], WqT[:].rearrange("(t p) c -> p t c", p=128))
        xt_pool0 = ctx.enter_context(tc.tile_pool(name="xt0", bufs=1))
        xts0 = xt_pool0.tile([128, 8, 512], FP16)
        nc.sync.dma_start(
            xts0[:], xT[:, 0:512].rearrange("(j p) n -> p j n", p=128))
        wob_t = singles.tile([128, 4, DIM], FP16)
        nc.sync.dma_start(wob_t[:], WoB[:].rearrange("(t p) c -> p t c", p=128))

        # ---------------- Pass A2: klrT/vlrT -> kbd/vbd -> M, dcorrT, vw ----
        kbd = a2sb.tile([128, 4, 128], FP16)
        nc.vector.memset(kbd[:], 0.0)
        vbd = a2sb.tile([128, 4, 128], FP16)
        nc.vector.memset(vbd[:], 0.0)
        m_sb = a2sb.tile([128, 8, QC], FP16)
        vw_sb = a2sb.tile([128, 4, DIM], FP16)
        expbias = a2sb.tile([128, 4], FP32)

        with tc.tile_pool(name="a2ps", bufs=2, space="PSUM") as a2ps:
            # transpose xE/xF: [64, 1024] -> 8 chunks of [128, 64] each
            xet_sb = a2sb.tile([128, 8, K], FP16)
            xft_sb = a2sb.tile([128, 8, K], FP16)
            for (src, dst) in ((xe16, xet_sb), (xf16, xft_sb)):
                tp = a2ps.tile([128, 8, K], FP16, tag="xt_ps")
                for j in range(8):
                    nc.tensor.transpose(
                        tp[:, j, :], src[:, j * 128:(j + 1) * 128],
                        ident_t[0:64, 0:64],
                    )
                nc.vector.tensor_copy(dst[:], tp[:])

            # klrT/vlrT tiles [c_pair, k] directly; + rank-1 bias on the copy
            for (w_t, xt, r1, dst) in (
                (wk_t, xet_sb, r1k_t, kbd),
                (wv_t, xft_sb, r1v_t, vbd),
            ):
                for t in range(4):
                    lp = a2ps.tile([128, K], FP32, tag="lr_ps")
                    for j in range(8):
                        nc.tensor.matmul(
                            lp[:], w_t[:, j, t * 128:(t + 1) * 128], xt[:, j, :],
                            start=(j == 0), stop=(j == 7),
                        )
                    nc.vector.tensor_add(out=dst[0:64, t, 0:64],
                                         in0=lp[0:64, :], in1=r1[0:64, t, :])
                    nc.vector.tensor_add(out=dst[64:128, t, 64:128],
                                         in0=lp[64:128, :], in1=r1[64:128, t, :])

            # bridge the wait for the wqt DMA (keeps the clock-gate warm)
            filler_mms(6)

            # M tiles: m_sb[p, j, hk] = M[j*128+p, hk] = (Wq klr^T)[d, hk]
            for j in range(8):
                m_ps = a2ps.tile([128, QC], FP32, tag="big")
                for t in range(4):
                    nc.tensor.matmul(
                        m_ps[:, t * 128:(t + 1) * 128],
                        wqt_t[:, t, j * 128:(j + 1) * 128],
                        kbd[:, t, :],
                        start=True, stop=True,
                    )
                if j % 2 == 0:
                    nc.vector.tensor_copy(m_sb[:, j, :], m_ps[:])
                else:
                    nc.scalar.copy(m_sb[:, j, :], m_ps[:])

            # dcorrT[hk] = bq . klr[hk-row]; exp bias = 0.125*dcorrT - 80
            dc_big = a2ps.tile([128, 512], FP32, tag="big")
            dc_ps = dc_big[:, 0:4]
            for t in range(4):
                nc.tensor.matmul(dc_ps[:, t:t + 1], kbd[:, t, :],
                                 bqp_t[:, t:t + 1], start=True, stop=True)
            nc.scalar.activation(out=expbias[:], in_=dc_ps[:],
                                 func=mybir.ActivationFunctionType.Copy,
                                 bias=-80.0, scale=0.125)

            # vw: pair-stacked (vlr_h @ Wout_h) in fp16
            for t in range(4):
                for f in (0, 512):
                    vw_ps = a2ps.tile([128, 512], FP32, tag="big")
                    nc.tensor.matmul(vw_ps[:], vbd[:, t, :],
                                     wob_t[:, t, f:f + 512],
                                     start=True, stop=True)
                    if f == 0:
                        nc.vector.tensor_copy(vw_sb[:, t, f:f + 512], vw_ps[:])
                    else:
                        nc.scalar.copy(vw_sb[:, t, f:f + 512], vw_ps[:])

        warm_cm.__exit__(None, None, None)

        # ---------------- Pass B: dotsT -> softmax -> out --------------------
        xt_pool = ctx.enter_context(tc.tile_pool(name="xt", bufs=3))
        exp_pool = ctx.enter_context(tc.tile_pool(name="expp", bufs=5))
        attn_pool = ctx.enter_context(tc.tile_pool(name="attn", bufs=2))
        osb_pool = ctx.enter_context(tc.tile_pool(name="osb", bufs=3))
        dots_pool = ctx.enter_context(tc.tile_pool(name="dots", bufs=2, space="PSUM"))
        sums_pool = ctx.enter_context(tc.tile_pool(name="sums", bufs=2, space="PSUM"))
        out_ps_pool = ctx.enter_context(tc.tile_pool(name="outps", bufs=2, space="PSUM"))

        xts_tiles = [None] * NSUPER
        attn_tiles = [None] * NSUPER

        def front_half(s):
            """dotsT -> exp -> sums -> attnT for superblock s."""
            if s == 0:
                xts = xts0
            else:
                xts = xt_pool.tile([128, 8, 512], FP16)
                nc.sync.dma_start(
                    xts[:],
                    xT[:, s * 512:(s + 1) * 512].rearrange("(j p) n -> p j n", p=128),
                )
            xts_tiles[s] = xts
            attnT = attn_pool.tile([128, 4, 512], FP16)
            attn_tiles[s] = attnT
            for t in range(4):
                dots_ps = dots_pool.tile([128, 512], FP32)
                for j in range(8):
                    nc.tensor.matmul(
                        dots_ps[:], m_sb[:, j, t * 128:(t + 1) * 128],
                        xts[:, j, :],
                        start=(j == 0), stop=(j == 7),
                    )
                expT = exp_pool.tile([128, 512], BF16)
                nc.scalar.activation(
                    out=expT[:], in_=dots_ps[:],
                    func=mybir.ActivationFunctionType.Exp,
                    scale=0.125, bias=expbias[:, t:t + 1],
                )
                sums_ps = sums_pool.tile([128, 512], FP32)
                nc.tensor.matmul(sums_ps[:], bones_t[:], expT[:],
                                 start=True, stop=True)
                rec = exp_pool.tile([128, 512], FP32, tag="rec")
                nc.vector.reciprocal_approx_fast(out=rec[:], in_=sums_ps[:])
                nc.vector.tensor_mul(out=attnT[:, t, :], in0=expT[:], in1=rec[:])

        def back_half(s):
            """attnT @ vw -> out rows for superblock s."""
            attnT = attn_tiles[s]
            for q in range(4):
                out_ps = out_ps_pool.tile([128, DIM], FP32)
                for f in (0, 512):
                    for t in range(4):
                        nc.tensor.matmul(
                            out_ps[:, f:f + 512],
                            attnT[:, t, q * 128:(q + 1) * 128],
                            vw_sb[:, t, f:f + 512],
                            start=(t == 0), stop=(t == 3),
                        )
                out_sb = osb_pool.tile([128, DIM], FP16)
                nc.vector.tensor_copy(out_sb[:, 0:512], out_ps[:, 0:512])
                nc.scalar.copy(out_sb[:, 512:1024], out_ps[:, 512:1024])
                i = s * 4 + q
                nc.gpsimd.dma_start(out_p[i * 128:(i + 1) * 128, :], out_sb[:])

        # software-pipeline by one superblock so the PE never waits on softmax
        front_half(0)
        for s in range(1, NSUPER):
            front_half(s)
            back_half(s - 1)
        back_half(NSUPER - 1)

    nc.finalize()
    _PROG_CACHE["nc"] = nc
    return nc


def shard_inputs(x, E, F, W_qkv, b_qkv, W_out, b_out):
    """Host-side prep: slice / transpose / cast per core."""
    x = np.asarray(x, dtype=np.float32)
    E = np.asarray(E, dtype=np.float32)
    F = np.asarray(F, dtype=np.float32)
    W_qkv = np.asarray(W_qkv, dtype=np.float32)
    b_qkv = np.asarray(b_qkv, dtype=np.float32)
    W_out = np.asarray(W_out, dtype=np.float32)

    sE = E.sum(0).astype(np.float32)   # [K]
    sF = F.sum(0).astype(np.float32)
    # EF pre-chunked: EFp[p, i*2K + k] = EF[i*128 + p, k]
    EF = np.concatenate([E, F], axis=1).astype(np.float16)       # [N, 2K]
    EFp = np.ascontiguousarray(
        EF.reshape(NCHUNK, 128, 2 * K).transpose(1, 0, 2).reshape(128, -1))

    ident = np.eye(128, dtype=np.float16)
    bones = np.zeros((128, 128), np.float32)
    bones[:64, :64] = 1.0
    bones[64:, 64:] = 1.0
    import ml_dtypes
    bones = bones.astype(ml_dtypes.bfloat16)

    in_maps = []
    xb_cache = {}
    for c in range(NCORES):
        b, hg = c // 2, c % 2
        hs = NH * hg
        if b not in xb_cache:
            xb16 = np.ascontiguousarray(x[:, b, :]).astype(np.float16)
            xT16 = np.ascontiguousarray(xb16.T)
            xb_cache[b] = (xb16, xT16)
        xb16, xT16 = xb_cache[b]

        qcols = slice(hs * DH, (hs + NH) * DH)
        kcols = slice(DIM + hs * DH, DIM + (hs + NH) * DH)
        vcols = slice(2 * DIM + hs * DH, 2 * DIM + (hs + NH) * DH)

        bq = b_qkv[qcols]                      # [512]
        bqp16 = np.ascontiguousarray(
            bq.reshape(4, 128).T).astype(np.float16)             # [128, 4]
        bk = b_qkv[kcols]
        bv = b_qkv[vcols]
        # r1kT[p, t*K + k] = bk[t*128 + p] * sE[k]
        r1kT = np.ascontiguousarray(
            (bk.reshape(4, 128)[:, :, None] * sE[None, None, :])
            .transpose(1, 0, 2).reshape(128, 4 * K))
        r1vT = np.ascontiguousarray(
            (bv.reshape(4, 128)[:, :, None] * sF[None, None, :])
            .transpose(1, 0, 2).reshape(128, 4 * K))

        in_maps.append({
            "ident": ident,
            "bones": bones,
            "EFp": EFp,
            "x_nat": xb16,
            "Wk": W_qkv[:, kcols].astype(np.float16),
            "Wv": W_qkv[:, vcols].astype(np.float16),
            "WqT": np.ascontiguousarray(W_qkv[:, qcols].T).astype(np.float16),
            "WoB": W_out[hs * DH:(hs + NH) * DH, :].astype(np.float16),
            "bqp": bqp16,
            "r1k": r1kT.astype(np.float32),
            "r1v": r1vT.astype(np.float32),
            "xT": xT16,
        })
    return in_maps


def kernel_impl(inputs, trace=False, **run_kwargs):
    nc = build_program()
    in_maps = shard_inputs(
        inputs["x"], inputs["E"], inputs["F"], inputs["W_qkv"],
        inputs["b_qkv"], inputs["W_out"], inputs["b_out"],
    )
    res = run_bass_kernel_spmd(nc, in_maps, list(range(NCORES)),
                               trace=trace, **run_kwargs)
    b_out = np.asarray(inputs["b_out"], dtype=np.float32)
    out = np.empty((N, B, DIM), np.float32)
    for b in range(B):
        out[:, b, :] = (res.results[2 * b]["out_p"].astype(np.float32)
                        + res.results[2 * b + 1]["out_p"].astype(np.float32)
                        + b_out)
    return out, res


def kernel(**inputs):
    out, _ = kernel_impl(inputs)
    return out
